# revision 10
# baseline (speedup 1.0000x reference)
"""Trainium2 Bass kernel for nn_CombinedRepeatCausalLinear (bf16, split-sums).

Math: out[r, t] = sum_{s<=t} x[r, s] * (w0[s]*dv0^(t-s) + w1[t]*dv1^(t-s)) + bias[t]

Computed transposed (t on partitions), data-parallel over the fused B*E
axis across 8 NeuronCores (r = 1024 rows per core). Everything on-device
is bf16; PSUM accumulation is fp32. The 2e-2 rel-err gate has ~4x margin
at bf16 (measured 4.7e-3).

Chunked linear-attention formulation, chunk L=128 along S (16 chunks),
with the chunk-sum reductions SPLIT in two groups so the second half of
the input loads (and its sum matmuls) overlap the first half's output
phase — keeping both the DMA pipe and the PE continuously busy:

  group A = chunks 0-7:  G_A matmuls accumulate S0_c,S1_c into PSUM rows
    1+2c, 2+2c (one standing bank per 512-wide r-half); after chunk 7 a
    [0:32]-row copy lands them in sall rows 0..31 (bf16).
  group B = chunks 8-15: same, but placed at rows 33+2(c-8) so the copy
    is the partition-aligned 32-row [32:64] slice into sall rows 32..63
    (engine ops with non-zero partition base are capped at 32 partitions).

  Output phase per chunk c and r-half (PSUM-fused, no separate add):
    psum  = D_c^T @ x_c          (start; upper-tri intra-chunk block)
    psum += M_c^T @ sall[0:K_c]  (stop; K=33 for c<=8, 64 after)
  then one [128,512] PSUM->SBUF bf16 copy per half (DVE/ACT split) into
  a 2-chunk staging tile, stored as 512 KB DMAs on alternating HWDGE
  rings. Chunks 0-8 only need sall rows < 33, so their output (and
  stores) overlap the loads + sums of chunks 9-15.

The host ships x^T pre-cast to bf16 in a chunk-tiled [128, 16*1024]
layout (every load/store per-partition contiguous) and un-permutes /
casts the bf16 result back to fp32.
"""

import sys

if "/opt/trn_rl_repo" not in sys.path:
    sys.path.insert(0, "/opt/trn_rl_repo")

import numpy as np
import ml_dtypes

import concourse.mybir as mybir
from concourse import bacc
from concourse.bass_utils import run_bass_kernel_spmd
from concourse.tile import TileContext

_P = 128
_B, _E, _S = 4, 2048, 2048
_NCORES = 8
_R = (_B * _E) // _NCORES  # 1024 rows (r) per core
_NCH = _S // _P  # 16 chunks of 128 along S
_NGA = 8  # chunks in sum-group A
# Group A uses sall rows 1..16 (bias at 0); group B starts at row 33 so
# its PSUM->SBUF copy is the partition-aligned, A-disjoint [32:66] slice.
_WA = 33  # group-A G width / cross-matmul K for chunks <= 8
_WB = 64  # total sall rows; group-B rows live at 33..48 (fit in [32:64])
_HALF = 512  # r per matmul (one PSUM bank, fp32)

_BF16 = mybir.dt.bfloat16
_F32 = mybir.dt.float32
_NPBF16 = ml_dtypes.bfloat16


def _g_off(c):
    """Column offset of chunk c's block in the packed G matrix."""
    return c * _WA if c < _NGA else _NGA * _WA + (c - _NGA) * _WB


def _g_w(c):
    return _WA if c < _NGA else _WB


_GW = _NGA * _WA + _NGA * _WB  # total packed-G columns


def _build_host_mats(w0, w1, dv0, dv1, bias):
    """Build D [128, 16*128], G [128, _GW], M [66, 16*128] (float64->bf16)."""
    w0 = w0.astype(np.float64)
    w1 = w1.astype(np.float64)
    bias = bias.astype(np.float64)
    s = np.arange(_P)[:, None]
    t = np.arange(_P)[None, :]
    mask = t >= s
    e = np.where(mask, t - s, 0).astype(np.float64)
    rev = np.arange(_P - 1, -1, -1).astype(np.float64)  # 127 - s

    D = np.zeros((_P, _NCH * _P), dtype=np.float64)
    G = np.zeros((_P, _GW), dtype=np.float64)
    M = np.zeros((_WB, _NCH * _P), dtype=np.float64)

    def rows_of(c):
        if c < _NGA:
            return 1 + 2 * c, 2 + 2 * c
        return _WA + 2 * (c - _NGA), _WA + 1 + 2 * (c - _NGA)

    for c in range(_NCH):
        base = c * _P
        blk = np.where(
            mask,
            w0[base : base + _P][:, None] * (dv0**e)
            + w1[base : base + _P][None, :] * (dv1**e),
            0.0,
        )
        D[:, c * _P : (c + 1) * _P] = blk
        r0, r1 = rows_of(c)
        G[:, _g_off(c) + r0] = dv1**rev
        G[:, _g_off(c) + r1] = w0[base : base + _P] * (dv0**rev)
        tg = base + np.arange(_P)
        M[0, c * _P : (c + 1) * _P] = bias[tg]
        for cp in range(c):
            e_cp = cp * _P + _P - 1
            m0, m1 = rows_of(cp)
            M[m0, c * _P : (c + 1) * _P] = w1[tg] * (dv1 ** (tg - e_cp))
            M[m1, c * _P : (c + 1) * _P] = dv0 ** (tg - e_cp)
    return D.astype(_NPBF16), G.astype(_NPBF16), M.astype(_NPBF16)


def _build(with_bias):
    nc = bacc.Bacc(
        "TRN2",
        target_bir_lowering=False,
        debug=False,
        enable_asserts=False,
        num_devices=_NCORES,
    )
    xt = nc.dram_tensor("xt", [_P, _NCH * _R], _BF16, kind="ExternalInput").ap()
    Dd = nc.dram_tensor("Dd", [_P, _NCH * _P], _BF16, kind="ExternalInput").ap()
    Gd = nc.dram_tensor("Gd", [_P, _GW], _BF16, kind="ExternalInput").ap()
    Md = nc.dram_tensor("Md", [_WB, _NCH * _P], _BF16, kind="ExternalInput").ap()
    outT = nc.dram_tensor("outT", [_P, _NCH * _R], _BF16, kind="ExternalOutput").ap()

    with TileContext(nc) as tc:
        with (
            tc.tile_pool(name="consts", bufs=1) as cpool,
            tc.tile_pool(name="xin", bufs=8) as xpool,
            tc.tile_pool(name="ot", bufs=3) as otpool,
            tc.tile_pool(name="ps", bufs=1, space="PSUM") as pspool,
            tc.tile_pool(name="po", bufs=6, space="PSUM") as popool,
        ):
            Gt = cpool.tile([_P, _GW], _BF16)
            Mt = cpool.tile([_WB, _NCH * _P], _BF16)
            Dt = cpool.tile([_P, _NCH * _P], _BF16)
            sall = cpool.tile([_WB, _R], _BF16)
            # clear sall so the never-copied rows (17..31, 49..65) can't
            # hold NaN garbage that 0-weight M rows would still poison
            nc.gpsimd.memset(sall[:], 0.0)
            # consts on the SWDGE queue -- a third DMA path in parallel
            # with the two HWDGE rings carrying x
            nc.gpsimd.dma_start(Gt[:], Gd[:])
            nc.gpsimd.dma_start(Dt[:], Dd[:])
            nc.gpsimd.dma_start(Mt[:], Md[:])

            # x: 8 slabs of 2 chunks (512 KB), alternating HWDGE rings
            xh = []
            for i in range(8):
                xs = xpool.tile([_P, 2 * _R], _BF16, tag="xh", name="xh")
                eng = nc.sync if i % 2 == 0 else nc.scalar
                eng.dma_start(xs[:], xt[:, i * 2 * _R : (i + 1) * 2 * _R])
                xh.append(xs)

            def xap(c, h):
                lo = (c % 2) * _R + h * _HALF
                return xh[c // 2][:, lo : lo + _HALF]

            psh = [
                pspool.tile([_WB, _HALF], _F32, tag="psA", name="psA"),
                pspool.tile([_WB, _HALF], _F32, tag="psB", name="psB"),
            ]

            def sums(c):
                first = c % _NGA == 0
                last = c % _NGA == _NGA - 1
                w = _g_w(c)
                off = _g_off(c)
                for h in (0, 1):
                    nc.tensor.matmul(
                        psh[h][0:w, :],
                        Gt[:, off : off + w],
                        xap(c, h),
                        start=first,
                        stop=last,
                    )

            po_of = {}

            def emit_diag(c):
                for h in (0, 1):
                    po = popool.tile([_P, _HALF], _F32, tag="po", name="po")
                    nc.tensor.matmul(
                        po[:], Dt[:, c * _P : (c + 1) * _P], xap(c, h),
                        start=True, stop=False,
                    )
                    po_of[(c, h)] = po

            ot = None

            def crossfin(c):
                nonlocal ot
                k = _WA if c <= _NGA else _WB
                if c % 2 == 0:
                    ot = otpool.tile([_P, 2 * _R], _BF16, tag="ot", name="ot")
                for h in (0, 1):
                    po = po_of.pop((c, h))
                    nc.tensor.matmul(
                        po[:],
                        Mt[0:k, c * _P : (c + 1) * _P],
                        sall[0:k, h * _HALF : (h + 1) * _HALF],
                        start=False,
                        stop=True,
                    )
                    dst = ot[:, (c % 2) * _R + h * _HALF : (c % 2) * _R + (h + 1) * _HALF]
                    if h == 0:
                        nc.vector.tensor_copy(dst, po[:])
                    else:
                        nc.scalar.copy(dst, po[:])
                if c % 2 == 1:
                    eng = nc.sync if (c // 2) % 2 == 0 else nc.scalar
                    eng.dma_start(outT[:, (c - 1) * _R : (c + 1) * _R], ot[:])

            # ---- sums A (chunks 0..7) ----
            for c in range(_NGA):
                sums(c)
            nc.vector.tensor_copy(sall[0:32, 0:_HALF], psh[0][0:32, :])
            nc.scalar.copy(sall[0:32, _HALF : 2 * _HALF], psh[1][0:32, :])
            if with_bias:
                nc.gpsimd.memset(sall[0:1, :], 1.0)

            # ---- output chunks 0..8 overlapped with sums B (8..15) ----
            emit_diag(0)
            emit_diag(1)
            for c in range(_NGA):
                sums(c + _NGA)
                crossfin(c)
                emit_diag(c + 2)
            crossfin(_NGA)  # chunk 8 still only needs sall rows < 33
            emit_diag(_NGA + 2)
            nc.vector.tensor_copy(sall[32:_WB, 0:_HALF], psh[0][32:_WB, :])
            nc.scalar.copy(sall[32:_WB, _HALF : 2 * _HALF], psh[1][32:_WB, :])

            # ---- output chunks 9..15 ----
            for c in range(_NGA + 1, _NCH):
                crossfin(c)
                if c + 2 < _NCH:
                    emit_diag(c + 2)
    nc.compile()
    return nc


def _shard_x(x):
    """x [B, E, S] fp32 -> per-core chunk-tiled x^T [128, NCH*R] bf16."""
    xf = np.asarray(x, dtype=np.float32).reshape(_B * _E, _S)
    xT = np.ascontiguousarray(xf.T)  # [S, B*E]
    shards = []
    for c in range(_NCORES):
        xc = xT[:, c * _R : (c + 1) * _R]  # [S, R]
        xc = np.ascontiguousarray(xc).reshape(_NCH, _P, _R).transpose(1, 0, 2)
        shards.append(np.ascontiguousarray(xc.astype(_NPBF16)).reshape(_P, _NCH * _R))
    return shards


def _unshard_out(parts):
    """per-core [128, NCH*R] bf16 -> [B, E, S] fp32."""
    cols = []
    for p in parts:
        pc = p.reshape(_P, _NCH, _R).transpose(1, 0, 2).reshape(_S, _R)
        cols.append(pc)
    outT = np.concatenate(cols, axis=1)  # [S, B*E] bf16
    return np.ascontiguousarray(outT.T).astype(np.float32).reshape(_B, _E, _S)


def _run(x, weight, bias, decay_value, trace=False):
    w = np.asarray(weight, dtype=np.float32)
    b = np.asarray(bias, dtype=np.float32)
    dv = np.asarray(decay_value, dtype=np.float32)
    dv0 = float(np.clip(dv[0, 0], 0.9, 1.0))
    dv1 = float(np.clip(dv[1, 0], 0.9, 1.0))

    D, G, M = _build_host_mats(w[0], w[1], dv0, dv1, b)
    nc = _build(bool(np.any(b)))

    shards = _shard_x(x)
    in_maps = [
        {"xt": shards[c], "Dd": D, "Gd": G, "Md": M} for c in range(_NCORES)
    ]

    res = run_bass_kernel_spmd(nc, in_maps, core_ids=list(range(_NCORES)), trace=trace)
    full = _unshard_out([res.results[c]["outT"] for c in range(_NCORES)])
    return full, res


def kernel(x, weight, bias, decay_value):
    full, _ = _run(x, weight, bias, decay_value, trace=False)
    return full


# revision 11
# speedup vs baseline: 1.0704x; 1.0704x over previous
"""Trainium2 Bass kernel for nn_CombinedRepeatCausalLinear (bf16, split-sums).

Math: out[r, t] = sum_{s<=t} x[r, s] * (w0[s]*dv0^(t-s) + w1[t]*dv1^(t-s)) + bias[t]

Computed transposed (t on partitions), data-parallel over the fused B*E
axis across 8 NeuronCores (r = 1024 rows per core). Everything on-device
is bf16; PSUM accumulation is fp32. The 2e-2 rel-err gate has ~4x margin
at bf16 (measured 4.7e-3).

Chunked linear-attention formulation, chunk L=128 along S (16 chunks),
with the chunk-sum reductions SPLIT in two groups so the second half of
the input loads (and its sum matmuls) overlap the first half's output
phase — keeping both the DMA pipe and the PE continuously busy:

  group A = chunks 0-7:  G_A matmuls accumulate S0_c,S1_c into PSUM rows
    1+2c, 2+2c (one standing bank per 512-wide r-half); after chunk 7 a
    [0:32]-row copy lands them in sall rows 0..31 (bf16).
  group B = chunks 8-15: same, but placed at rows 33+2(c-8) so the copy
    is the partition-aligned 32-row [32:64] slice into sall rows 32..63
    (engine ops with non-zero partition base are capped at 32 partitions).

  Output phase per chunk c and r-half (PSUM-fused, no separate add):
    psum  = D_c^T @ x_c          (start; upper-tri intra-chunk block)
    psum += M_c^T @ sall[0:K_c]  (stop; K=32 for c<=8, 64 after --
                                  chunks <=8 only need rows 0..16)
  then one [128,512] PSUM->SBUF bf16 copy per half (DVE/ACT split) into
  a 2-chunk staging tile, stored as 512 KB DMAs on alternating HWDGE
  rings. Chunks 0-8 only need sall rows < 33, so their output (and
  stores) overlap the loads + sums of chunks 9-15.

The host ships x^T pre-cast to bf16 in a chunk-tiled [128, 16*1024]
layout (every load/store per-partition contiguous) and un-permutes /
casts the bf16 result back to fp32.
"""

import sys

if "/opt/trn_rl_repo" not in sys.path:
    sys.path.insert(0, "/opt/trn_rl_repo")

import numpy as np
import ml_dtypes

import concourse.mybir as mybir
from concourse import bacc
from concourse.bass_utils import run_bass_kernel_spmd
from concourse.tile import TileContext

_P = 128
_B, _E, _S = 4, 2048, 2048
_NCORES = 8
_R = (_B * _E) // _NCORES  # 1024 rows (r) per core
_NCH = _S // _P  # 16 chunks of 128 along S
_NGA = 8  # chunks in sum-group A
# Group A uses sall rows 1..16 (bias at 0); group B starts at row 33 so
# its PSUM->SBUF copy is the partition-aligned, A-disjoint [32:66] slice.
_WA = 33  # group-A G width / cross-matmul K for chunks <= 8
_WB = 64  # total sall rows; group-B rows live at 33..48 (fit in [32:64])
_HALF = 512  # r per matmul (one PSUM bank, fp32)

_BF16 = mybir.dt.bfloat16
_F32 = mybir.dt.float32
_NPBF16 = ml_dtypes.bfloat16


def _g_off(c):
    """Column offset of chunk c's block in the packed G matrix."""
    return c * _WA if c < _NGA else _NGA * _WA + (c - _NGA) * _WB


def _g_w(c):
    return _WA if c < _NGA else _WB


_GW = _NGA * _WA + _NGA * _WB  # total packed-G columns


def _build_host_mats(w0, w1, dv0, dv1, bias):
    """Build D [128, 16*128], G [128, _GW], M [66, 16*128] (float64->bf16)."""
    w0 = w0.astype(np.float64)
    w1 = w1.astype(np.float64)
    bias = bias.astype(np.float64)
    s = np.arange(_P)[:, None]
    t = np.arange(_P)[None, :]
    mask = t >= s
    e = np.where(mask, t - s, 0).astype(np.float64)
    rev = np.arange(_P - 1, -1, -1).astype(np.float64)  # 127 - s

    D = np.zeros((_P, _NCH * _P), dtype=np.float64)
    G = np.zeros((_P, _GW), dtype=np.float64)
    M = np.zeros((_WB, _NCH * _P), dtype=np.float64)

    def rows_of(c):
        if c < _NGA:
            return 1 + 2 * c, 2 + 2 * c
        return _WA + 2 * (c - _NGA), _WA + 1 + 2 * (c - _NGA)

    for c in range(_NCH):
        base = c * _P
        blk = np.where(
            mask,
            w0[base : base + _P][:, None] * (dv0**e)
            + w1[base : base + _P][None, :] * (dv1**e),
            0.0,
        )
        D[:, c * _P : (c + 1) * _P] = blk
        r0, r1 = rows_of(c)
        G[:, _g_off(c) + r0] = dv1**rev
        G[:, _g_off(c) + r1] = w0[base : base + _P] * (dv0**rev)
        tg = base + np.arange(_P)
        M[0, c * _P : (c + 1) * _P] = bias[tg]
        for cp in range(c):
            e_cp = cp * _P + _P - 1
            m0, m1 = rows_of(cp)
            M[m0, c * _P : (c + 1) * _P] = w1[tg] * (dv1 ** (tg - e_cp))
            M[m1, c * _P : (c + 1) * _P] = dv0 ** (tg - e_cp)
    return D.astype(_NPBF16), G.astype(_NPBF16), M.astype(_NPBF16)


def _build(with_bias):
    nc = bacc.Bacc(
        "TRN2",
        target_bir_lowering=False,
        debug=False,
        enable_asserts=False,
        num_devices=_NCORES,
    )
    xt = nc.dram_tensor("xt", [_P, _NCH * _R], _BF16, kind="ExternalInput").ap()
    Dd = nc.dram_tensor("Dd", [_P, _NCH * _P], _BF16, kind="ExternalInput").ap()
    Gd = nc.dram_tensor("Gd", [_P, _GW], _BF16, kind="ExternalInput").ap()
    Md = nc.dram_tensor("Md", [_WB, _NCH * _P], _BF16, kind="ExternalInput").ap()
    outT = nc.dram_tensor("outT", [_P, _NCH * _R], _BF16, kind="ExternalOutput").ap()

    with TileContext(nc) as tc:
        with (
            tc.tile_pool(name="consts", bufs=1) as cpool,
            tc.tile_pool(name="xin", bufs=8) as xpool,
            tc.tile_pool(name="ot", bufs=3) as otpool,
            tc.tile_pool(name="ps", bufs=1, space="PSUM") as pspool,
            tc.tile_pool(name="po", bufs=6, space="PSUM") as popool,
        ):
            Gt = cpool.tile([_P, _GW], _BF16)
            Mt = cpool.tile([_WB, _NCH * _P], _BF16)
            Dt = cpool.tile([_P, _NCH * _P], _BF16)
            sall = cpool.tile([_WB, _R], _BF16)
            # PE warm-up: the HAM clock gate holds the PE at 1.2 GHz until
            # ~3.4us of sustained activity; burn that window on dummy
            # matmuls (reading a memset tile, closed psum groups into a
            # sums bank that the first real start=True clears) so the real
            # stream runs at 2.4 GHz from the start.
            dsrc = cpool.tile([_P, _P], _BF16)
            nc.gpsimd.memset(dsrc[:], 0.0)

            # x: 8 slabs of 2 chunks (512 KB) + consts, interleaved on the
            # two HWDGE rings so each lands just before its first use
            xh = [
                xpool.tile([_P, 2 * _R], _BF16, tag="xh", name=f"xh{i}")
                for i in range(8)
            ]

            def ldx(i, eng):
                eng.dma_start(xh[i][:], xt[:, i * 2 * _R : (i + 1) * 2 * _R])

            GA = _NGA * _WA  # G cols for group A
            MA = (_NGA + 1) * _P  # M cols for chunks 0..8
            nc.sync.dma_start(Gt[:, 0:GA], Gd[:, 0:GA])
            ldx(0, nc.sync)
            ldx(1, nc.scalar)
            nc.scalar.dma_start(Gt[:, GA:_GW], Gd[:, GA:_GW])
            ldx(2, nc.sync)
            ldx(3, nc.scalar)
            nc.sync.dma_start(Dt[:, 0 : _NGA * _P], Dd[:, 0 : _NGA * _P])
            nc.scalar.dma_start(Mt[:, 0:MA], Md[:, 0:MA])
            ldx(4, nc.sync)
            ldx(5, nc.scalar)
            nc.scalar.dma_start(Dt[:, _NGA * _P :], Dd[:, _NGA * _P :])
            ldx(6, nc.sync)
            ldx(7, nc.scalar)
            nc.scalar.dma_start(Mt[:, MA:], Md[:, MA:])

            def xap(c, h):
                lo = (c % 2) * _R + h * _HALF
                return xh[c // 2][:, lo : lo + _HALF]

            psh = [
                pspool.tile([_WB, _HALF], _F32, tag="psA", name="psA"),
                pspool.tile([_WB, _HALF], _F32, tag="psB", name="psB"),
            ]
            for _ in range(14):
                nc.tensor.matmul(
                    psh[0][0:_WB, 0:_P],
                    dsrc[:, 0:_WB],
                    dsrc[:],
                    start=True,
                    stop=True,
                )

            def sums(c):
                first = c % _NGA == 0
                last = c % _NGA == _NGA - 1
                w = _g_w(c)
                off = _g_off(c)
                for h in (0, 1):
                    nc.tensor.matmul(
                        psh[h][0:w, :],
                        Gt[:, off : off + w],
                        xap(c, h),
                        start=first,
                        stop=last,
                    )

            po_of = {}

            def emit_diag(c):
                for h in (0, 1):
                    po = popool.tile([_P, _HALF], _F32, tag="po", name="po")
                    nc.tensor.matmul(
                        po[:], Dt[:, c * _P : (c + 1) * _P], xap(c, h),
                        start=True, stop=False,
                    )
                    po_of[(c, h)] = po

            ot = None

            def crossfin(c):
                nonlocal ot
                k = 32 if c <= _NGA else _WB
                if c % 2 == 0:
                    ot = otpool.tile([_P, 2 * _R], _BF16, tag="ot", name="ot")
                for h in (0, 1):
                    po = po_of.pop((c, h))
                    nc.tensor.matmul(
                        po[:],
                        Mt[0:k, c * _P : (c + 1) * _P],
                        sall[0:k, h * _HALF : (h + 1) * _HALF],
                        start=False,
                        stop=True,
                    )
                    dst = ot[:, (c % 2) * _R + h * _HALF : (c % 2) * _R + (h + 1) * _HALF]
                    if h == 0:
                        nc.vector.tensor_copy(dst, po[:])
                    else:
                        nc.scalar.copy(dst, po[:])
                if c % 2 == 1:
                    eng = nc.sync if (c // 2) % 2 == 0 else nc.scalar
                    eng.dma_start(outT[:, (c - 1) * _R : (c + 1) * _R], ot[:])

            # ---- sums A (chunks 0..7) ----
            for c in range(_NGA):
                sums(c)
            nc.vector.tensor_copy(sall[0:32, 0:_HALF], psh[0][0:32, :])
            nc.scalar.copy(sall[0:32, _HALF : 2 * _HALF], psh[1][0:32, :])
            if with_bias:
                nc.gpsimd.memset(sall[0:1, :], 1.0)

            # ---- output chunks 0..8 overlapped with sums B (8..15) ----
            emit_diag(0)
            emit_diag(1)
            for c in range(_NGA):
                sums(c + _NGA)
                crossfin(c)
                emit_diag(c + 2)
            crossfin(_NGA)  # chunk 8 still only needs sall rows < 33
            emit_diag(_NGA + 2)
            nc.vector.tensor_copy(sall[32:_WB, 0:_HALF], psh[0][32:_WB, :])
            nc.scalar.copy(sall[32:_WB, _HALF : 2 * _HALF], psh[1][32:_WB, :])

            # ---- output chunks 9..15 ----
            for c in range(_NGA + 1, _NCH):
                crossfin(c)
                if c + 2 < _NCH:
                    emit_diag(c + 2)
    nc.compile()
    return nc


def _shard_x(x):
    """x [B, E, S] fp32 -> per-core chunk-tiled x^T [128, NCH*R] bf16."""
    xf = np.asarray(x, dtype=np.float32).reshape(_B * _E, _S)
    xT = np.ascontiguousarray(xf.T)  # [S, B*E]
    shards = []
    for c in range(_NCORES):
        xc = xT[:, c * _R : (c + 1) * _R]  # [S, R]
        xc = np.ascontiguousarray(xc).reshape(_NCH, _P, _R).transpose(1, 0, 2)
        shards.append(np.ascontiguousarray(xc.astype(_NPBF16)).reshape(_P, _NCH * _R))
    return shards


def _unshard_out(parts):
    """per-core [128, NCH*R] bf16 -> [B, E, S] fp32."""
    cols = []
    for p in parts:
        pc = p.reshape(_P, _NCH, _R).transpose(1, 0, 2).reshape(_S, _R)
        cols.append(pc)
    outT = np.concatenate(cols, axis=1)  # [S, B*E] bf16
    return np.ascontiguousarray(outT.T).astype(np.float32).reshape(_B, _E, _S)


def _run(x, weight, bias, decay_value, trace=False):
    w = np.asarray(weight, dtype=np.float32)
    b = np.asarray(bias, dtype=np.float32)
    dv = np.asarray(decay_value, dtype=np.float32)
    dv0 = float(np.clip(dv[0, 0], 0.9, 1.0))
    dv1 = float(np.clip(dv[1, 0], 0.9, 1.0))

    D, G, M = _build_host_mats(w[0], w[1], dv0, dv1, b)
    nc = _build(bool(np.any(b)))

    shards = _shard_x(x)
    in_maps = [
        {"xt": shards[c], "Dd": D, "Gd": G, "Md": M} for c in range(_NCORES)
    ]

    res = run_bass_kernel_spmd(nc, in_maps, core_ids=list(range(_NCORES)), trace=trace)
    full = _unshard_out([res.results[c]["outT"] for c in range(_NCORES)])
    return full, res


def kernel(x, weight, bias, decay_value):
    full, _ = _run(x, weight, bias, decay_value, trace=False)
    return full
